# revision 38
# baseline (speedup 1.0000x reference)
"""Trainium2 Bass kernel for nn_NTPLayer (GNN message passing layer).

Sharding: nodes (and their contiguous outgoing-edge groups; e0 is sorted)
across 8 cores.  Per-core edge slots are packed so that every source-node
group fits entirely inside one 128-edge tile; attention never crosses
tile boundaries.

v2 design (all matmuls bf16):
  A) gather x[e0],x[e1] (transposed, bf16) -> edge MLP (Wc+gelu) ->
     q/k/v projections (DVE bias adds, scale/bv folded host-side) ->
     per-tile masked attention (host-precomputed additive mask, one exp
     per tile, DVE softmax normalize) -> attention-out via P^T transposes
     -> o-proj + gelu -> per-head gate logits -> segment softmax over e0
     groups (host-precomputed indicator matmuls) -> contrib rows (bf16)
     -> scratch DRAM.  Scalar-engine work is batched into one Gelu block
     + one Exp block per chunk (software-pipelined across chunks) to
     avoid activation-table thrash.
  S) transposed dma_gather of contrib rows in e1-sorted, window-padded
     order (coalesced SWDGE packets) -> PE transpose back to edge-major
     -> dest-window indicator matmuls accumulate dense per-window sums
     -> static writes into partial[8000,128] (bf16).  No scatter-add,
     no zero-init.
  R) ReduceScatter(add, bf16) -> this core's [1000,128] shard -> out.
"""

import numpy as np
import ml_dtypes

import concourse.bass as bass
import concourse.bacc as bacc
import concourse.mybir as mybir
import concourse.tile as tile
from concourse.bass_utils import run_bass_kernel_spmd
from concourse.masks import make_identity

dt = mybir.dt
F32 = dt.float32
BF16 = dt.bfloat16
I16 = dt.int16

N = 8000
DIN = 128
DOUT = 256
H = 4
DH = 64
MAXD = 32
NCORES = 8
NPC = N // NCORES          # nodes per core
TW = 128                   # edge-slots per attention tile
TB = 512                   # slots per chunk (4 tiles)
WIN = 128                  # dest-node window (phase S)
NW = (N + WIN - 1) // WIN  # 63 windows (last one 64 rows)
SH = 1024                  # output rows incl pad
NEG = -1.0e30

TRACE = [False]
DEBUG = [None]   # "scratch" | "y1d" -> expose that internal tensor as output
LAST_PROFILE = {}

BF = ml_dtypes.bfloat16


# ----------------------------------------------------------------------------
# host-side preprocessing
# ----------------------------------------------------------------------------

def _pack_groups_ffd(sizes, cap):
    """First-fit-decreasing bin packing.  Returns list of lists of group
    indices per bin (groups may be assigned to any bin)."""
    order = np.argsort(-np.asarray(sizes), kind="stable")
    bins, fills = [], []
    for gi in order:
        s = int(sizes[gi])
        assert s <= cap
        for b in range(len(bins)):
            if fills[b] + s <= cap:
                bins[b].append(int(gi))
                fills[b] += s
                break
        else:
            bins.append([int(gi)])
            fills.append(s)
    return bins


def _idx16(idx, n_pad):
    """dma_gather idx layout: [128, n_pad//16] int16, idx i at
    [i%16, i//16], 16-partition pattern replicated to 128 partitions."""
    a = np.full(n_pad, 0, np.int64)
    a[: len(idx)] = idx
    a = a.reshape(-1, 16).T.astype(np.int16)
    return np.tile(a, (8, 1))


def _prep_core_a(e0, e1, lo, hi):
    """Phase-A slot layout: pack e0-groups into 128-slot tiles."""
    els = int(np.searchsorted(e0, lo, side="left"))
    ele = int(np.searchsorted(e0, hi, side="left"))
    le0 = e0[els:ele]
    m = ele - els

    if m > 0:
        gnodes, gsizes = np.unique(le0, return_counts=True)
    else:
        gnodes, gsizes = np.array([], np.int64), np.array([], np.int64)
    bins = _pack_groups_ffd(gsizes, TW)

    nslots = len(bins) * TW
    slot_e0 = np.zeros(nslots, np.int64)
    slot_e1 = np.full(nslots, -1, np.int64)
    slot_gidr = np.full(nslots, -1.0, np.float32)
    gstart = np.concatenate([[0], np.cumsum(gsizes)]).astype(np.int64)
    for t, glist in enumerate(bins):
        p = t * TW
        for gr, g in enumerate(glist):
            sz = int(gsizes[g])
            slot_e0[p : p + sz] = gnodes[g]
            e_sl = slice(els + int(gstart[g]), els + int(gstart[g]) + sz)
            slot_e1[p : p + sz] = e1[e_sl]
            slot_gidr[p : p + sz] = gr
            p += sz
    return dict(slot_e0=slot_e0, slot_e1=slot_e1, slot_gidr=slot_gidr)


def _prepare(x, Wc, bc, Wq, bq, Wk, bk, Wv, bv, Wo, bo, e0, e1):
    e0 = np.asarray(e0, np.int64)
    e1 = np.asarray(e1, np.int64)
    cores = [_prep_core_a(e0, e1, k * NPC, (k + 1) * NPC) for k in range(NCORES)]

    E_pad = max(len(c["slot_e1"]) for c in cores)
    E_pad = -(-E_pad // TB) * TB
    NCH = E_pad // TB
    NT = NCH * 4

    # ---- phase S: e1-sorted slots, window-padded; tile counts shared
    win_slots = []
    for c in cores:
        se1 = c["slot_e1"]
        real = np.nonzero(se1 >= 0)[0]
        order = real[np.argsort(se1[real], kind="stable")]
        dvals = se1[order]
        per_w = []
        for w in range(NW):
            lo_ = np.searchsorted(dvals, w * WIN, side="left")
            hi_ = np.searchsorted(dvals, min((w + 1) * WIN, N), side="left")
            per_w.append(order[lo_:hi_])
        win_slots.append(per_w)
    Tws = [max(1, max(-(-len(win_slots[k][w]) // TW) for k in range(NCORES)))
           for w in range(NW)]
    NST = sum(Tws)
    ES_pad = NST * TW
    NSCH = -(-NST // 4)

    def padto(a, n, fill):
        out = np.full(n, fill, a.dtype)
        out[: len(a)] = a
        return out

    def padto_2d_x(x_, lo):
        out = np.zeros((SH, DIN), BF)
        out[:NPC] = x_[lo : lo + NPC].astype(BF)
        return out

    iota128 = np.arange(TW)
    in_maps = []
    for k in range(NCORES):
        c = cores[k]
        gidr = padto(c["slot_gidr"], E_pad, -1.0)
        e0s = padto(c["slot_e0"], E_pad, 0)
        e1s = padto(np.maximum(c["slot_e1"], 0), E_pad, 0)

        # per-chunk unique-source gather (y1 rows) + src-broadcast indicator
        lo = k * NPC
        e0loc = np.where(gidr >= 0, e0s - lo, -1)      # local src, -1 pads
        y1idx = np.zeros((NCH, TW), np.int64)
        gsd = np.zeros((NCH, TW, TB), BF)
        for cc in range(NCH):
            seg = e0loc[cc * TB : (cc + 1) * TB]
            uniq = np.unique(seg[seg >= 0])
            assert len(uniq) <= TW
            sent = np.full(TW, -1, np.int64)
            sent[: len(uniq)] = uniq
            y1idx[cc] = np.maximum(sent, 0)
            gsd[cc] = ((sent[:, None] == seg[None, :])
                       & (sent[:, None] >= 0)).astype(BF)

        g2 = gidr.reshape(NT, TW)
        madd_ = np.where(g2[:, :, None] == g2[:, None, :], 0.0, NEG
                         ).astype(np.float32)                          # [NT,q,k]
        ind_ = (g2[:, :, None] == iota128[None, None, :]).astype(BF)   # [NT,e,g]
        indT_ = np.ascontiguousarray(np.transpose(ind_, (0, 2, 1)))    # [NT,g,e]

        sgx = np.zeros(ES_pad, np.int64)
        drel = np.full(ES_pad, -1, np.int64)
        p = 0
        for w in range(NW):
            sl = win_slots[k][w]
            sgx[p : p + len(sl)] = sl
            drel[p : p + len(sl)] = c["slot_e1"][sl] - w * WIN
            p += Tws[w] * TW
        assert p == ES_pad
        sindw_ = (drel.reshape(NST, TW)[:, :, None]
                  == iota128[None, None, :]).astype(BF)                # [NST,s,d]

        in_maps.append(dict(
            xloc=padto_2d_x(x, lo),
            y1i=_idx16(y1idx.reshape(-1), NCH * TW),
            gsd=gsd,
            e1i=_idx16(e1s, E_pad),
            madd=np.ascontiguousarray(
                madd_.reshape(NCH, 4, TW, TW).transpose(0, 2, 1, 3)),
            indd=np.ascontiguousarray(
                ind_.reshape(NCH, 4, TW, TW).transpose(0, 2, 1, 3)),
            indt=np.ascontiguousarray(
                indT_.reshape(NCH, 4, TW, TW).transpose(0, 2, 1, 3)),
            sgi=_idx16(sgx, ES_pad),
            sindw=sindw_,
        ))

    dims = dict(E_pad=E_pad, NCH=NCH, NST=NST, NSCH=NSCH, ES_pad=ES_pad,
                Tws=Tws)

    # shared tensors; fold 1/sqrt(dh) into Wq/bq and bv@Wo.T+bo into bo'
    scale = 1.0 / np.sqrt(DH)
    bo_f = (bv.astype(np.float64) @ Wo.T.astype(np.float64)
            + bo.astype(np.float64)).astype(np.float32)
    shared = dict(
        x16=np.ascontiguousarray(x.astype(BF)),
        wct=np.ascontiguousarray(Wc.T.astype(BF)),
        wqt=np.ascontiguousarray((scale * Wq).T.astype(BF)),
        wkt=np.ascontiguousarray(Wk.T.astype(BF)),
        wvt=np.ascontiguousarray(Wv.T.astype(BF)),
        wot=np.ascontiguousarray(Wo.T.astype(BF)),
        bc2=np.ascontiguousarray(bc.reshape(2, 128).T.astype(np.float32)),
        bq2=np.ascontiguousarray(
            (scale * bq).reshape(2, 128).T.astype(np.float32)),
        bk2=np.ascontiguousarray(bk.reshape(2, 128).T.astype(np.float32)),
        bor=np.ascontiguousarray(bo_f.reshape(1, 256).astype(BF)),
    )
    for m in in_maps:
        m.update(shared)
    return in_maps, dims


# ----------------------------------------------------------------------------
# device kernel
# ----------------------------------------------------------------------------

def _build(dims):
    E_pad, NCH, NST, ES_pad = (
        dims["E_pad"], dims["NCH"], dims["NST"], dims["ES_pad"])

    nc = bacc.Bacc(None, target_bir_lowering=False)

    x16 = nc.dram_tensor("x16", [N, DIN], BF16, kind="ExternalInput")
    wct = nc.dram_tensor("wct", [256, 256], BF16, kind="ExternalInput")
    wqt = nc.dram_tensor("wqt", [256, 256], BF16, kind="ExternalInput")
    wkt = nc.dram_tensor("wkt", [256, 256], BF16, kind="ExternalInput")
    wvt = nc.dram_tensor("wvt", [256, 256], BF16, kind="ExternalInput")
    wot = nc.dram_tensor("wot", [256, 256], BF16, kind="ExternalInput")
    bc2 = nc.dram_tensor("bc2", [128, 2], F32, kind="ExternalInput")
    bq2 = nc.dram_tensor("bq2", [128, 2], F32, kind="ExternalInput")
    bk2 = nc.dram_tensor("bk2", [128, 2], F32, kind="ExternalInput")
    bor = nc.dram_tensor("bor", [1, 256], BF16, kind="ExternalInput")
    xloc = nc.dram_tensor("xloc", [SH, DIN], BF16, kind="ExternalInput")
    y1i = nc.dram_tensor("y1i", [128, NCH * TW // 16], I16, kind="ExternalInput")
    gsd = nc.dram_tensor("gsd", [NCH, TW, TB], BF16, kind="ExternalInput")
    e1i = nc.dram_tensor("e1i", [128, E_pad // 16], I16, kind="ExternalInput")
    madd = nc.dram_tensor("madd", [NCH, TW, 4, TW], F32, kind="ExternalInput")
    indd = nc.dram_tensor("indd", [NCH, TW, 4, TW], BF16, kind="ExternalInput")
    indt = nc.dram_tensor("indt", [NCH, TW, 4, TW], BF16, kind="ExternalInput")
    sgi = nc.dram_tensor("sgi", [128, ES_pad // 16], I16, kind="ExternalInput")
    sindw = nc.dram_tensor("sindw", [NST, TW, TW], BF16, kind="ExternalInput")

    outp = nc.dram_tensor("out", [SH, DIN], F32, kind="ExternalOutput")

    scratch = nc.dram_tensor(
        "scratch", [E_pad, 128], BF16,
        kind="ExternalOutput" if DEBUG[0] == "scratch" else "Internal")
    dbgf = (nc.dram_tensor("dbgf", [NCH, 128, 2048], F32,
                           kind="ExternalOutput")
            if DEBUG[0] in ("sm", "posb", "hsb") else None)
    dbgh = (nc.dram_tensor("dbgh", [NCH, 128, 2048], BF16,
                           kind="ExternalOutput")
            if DEBUG[0] in ("pexp", "ex", "q", "k") else None)
    y1d = nc.dram_tensor(
        "y1d", [SH, DOUT], BF16,
        kind="ExternalOutput" if DEBUG[0] == "y1d" else "Internal")
    partial = nc.dram_tensor("partial", [N, 128], BF16)
    rsout = nc.dram_tensor("rsout", [NPC, 128], BF16)

    with tile.TileContext(nc) as tc:
        _body(nc, tc, locals(), dims)
    nc.finalize()
    return nc


def _body(nc, tc, T, dims):
    E_pad, NCH, NST, NSCH, ES_pad, Tws = (
        dims["E_pad"], dims["NCH"], dims["NST"], dims["NSCH"],
        dims["ES_pad"], dims["Tws"])
    AF = mybir.ActivationFunctionType
    OP = mybir.AluOpType
    x16, wct, wqt, wkt, wvt, wot = (
        T["x16"], T["wct"], T["wqt"], T["wkt"], T["wvt"], T["wot"])
    bc2, bq2, bk2, bor = T["bc2"], T["bq2"], T["bk2"], T["bor"]
    xloc, y1i, gsd, e1i, madd, indd, indt = (
        T["xloc"], T["y1i"], T["gsd"], T["e1i"], T["madd"], T["indd"],
        T["indt"])
    sgi, sindw = T["sgi"], T["sindw"]
    outp, scratch, y1d, partial, rsout = (
        T["outp"], T["scratch"], T["y1d"], T["partial"], T["rsout"])

    import contextlib
    ctx = contextlib.ExitStack()
    with ctx:
        cpool = ctx.enter_context(tc.tile_pool(name="const", bufs=1))
        identf = cpool.tile([128, 128], F32)
        make_identity(nc, identf[:])
        ident = cpool.tile([128, 128], BF16)
        nc.vector.tensor_copy(ident[:], identf[:])
        ones1 = cpool.tile([1, 128], BF16)
        nc.gpsimd.memset(ones1[:], 1.0)

        wct_s = cpool.tile([128, 2, 256], BF16)
        nc.sync.dma_start(wct_s[:], wct[:].rearrange("(i p) o -> p i o", p=128))
        wqt_s = cpool.tile([128, 2, 256], BF16)
        nc.sync.dma_start(wqt_s[:], wqt[:].rearrange("(i p) o -> p i o", p=128))
        wkt_s = cpool.tile([128, 2, 256], BF16)
        nc.sync.dma_start(wkt_s[:], wkt[:].rearrange("(i p) o -> p i o", p=128))
        wvt_s = cpool.tile([128, 2, 256], BF16)
        nc.sync.dma_start(wvt_s[:], wvt[:].rearrange("(i p) o -> p i o", p=128))
        wot_s = cpool.tile([64, 4, 256], BF16)
        nc.sync.dma_start(wot_s[:], wot[:].rearrange("(i p) o -> p i o", p=64))
        bc_s = cpool.tile([128, 2], F32)
        nc.sync.dma_start(bc_s[:], bc2[:])
        bq_s = cpool.tile([128, 2], F32)
        nc.sync.dma_start(bq_s[:], bq2[:])
        bk_s = cpool.tile([128, 2], F32)
        nc.sync.dma_start(bk_s[:], bk2[:])
        bo_s = cpool.tile([1, 256], BF16)
        nc.sync.dma_start(bo_s[:], bor[:])
        y1i_s = cpool.tile([128, NCH * TW // 16], I16)
        nc.sync.dma_start(y1i_s[:], y1i[:])
        e1i_s = cpool.tile([128, E_pad // 16], I16)
        nc.sync.dma_start(e1i_s[:], e1i[:])
        sgi_s = cpool.tile([128, ES_pad // 16], I16)
        nc.sync.dma_start(sgi_s[:], sgi[:])

        # pre-zeroed khz double buffers [128, 2(heads), TB] per feature-half;
        # live 64-row halves rewritten per chunk, zero halves persist.
        khz_bufs = []
        for b in range(2):
            pair = []
            for f in range(2):
                t_ = cpool.tile([128, 2, TB], BF16, tag=f"khz{b}{f}")
                nc.gpsimd.memset(t_[:], 0.0)
                pair.append(t_)
            khz_bufs.append(pair)

        r512 = nc.alloc_register(mybir.EngineType.Pool, "n512")
        nc.gpsimd.reg_mov(r512, TB)
        r128 = nc.alloc_register(mybir.EngineType.Pool, "n128")
        nc.gpsimd.reg_mov(r128, TW)
        last_n = (NST - (NSCH - 1) * 4) * TW
        rlast = nc.alloc_register(mybir.EngineType.Pool, "nlast")
        nc.gpsimd.reg_mov(rlast, last_n)

        gat = ctx.enter_context(tc.tile_pool(name="gat", bufs=2))
        act = ctx.enter_context(tc.tile_pool(name="act", bufs=2))
        til = ctx.enter_context(tc.tile_pool(name="til", bufs=2))
        stt = ctx.enter_context(tc.tile_pool(name="stt", bufs=2))
        # PSUM: 8 banks of [128, 2KB].  big0+big1 (pex/pq/pk/v, bufs=1) = 2,
        # s (scores, bufs=2) = 2, ppt (transposes, 4 regions) = 1,
        # pvo (4 regions) = 1, po (2 regions) = 1, smt (pdn+prd regions) = 1.
        psA = contextlib.ExitStack()
        ctx.enter_context(psA)
        ps_big = psA.enter_context(tc.tile_pool(name="psbig", bufs=1, space="PSUM"))
        ps_s = psA.enter_context(tc.tile_pool(name="pss", bufs=1, space="PSUM"))
        ps_t = psA.enter_context(tc.tile_pool(name="pst", bufs=2, space="PSUM"))
        ps_pv = psA.enter_context(tc.tile_pool(name="pspv", bufs=2, space="PSUM"))
        ps_o = psA.enter_context(tc.tile_pool(name="pso", bufs=1, space="PSUM"))

        # ------------------------------------------------------------------
        # y1 = x_local @ Wc1.T per local node, written to y1d (DRAM bf16).
        # Per chunk we then gather <=128 unique-source y1 rows and broadcast
        # them to slots with an indicator matmul (no 512-row src gather).
        # ------------------------------------------------------------------
        xls = cpool.tile([128, 8, 128], BF16)
        nc.sync.dma_start(xls[:], xloc[:].rearrange("(a p) d -> p a d", p=128))
        for a in range(8):
            pxt = ps_t.tile([128, H, 128], BF16, tag="ppt")
            nc.tensor.transpose(pxt[:, 0, :], xls[:, a, :], ident[:])
            xts = til.tile([128, 128], BF16, tag="xts")
            nc.scalar.activation(xts[:], pxt[:, 0, :], AF.Identity)
            py1 = ps_o.tile([128, 2, 256], F32, tag="po")
            nc.tensor.matmul(py1[:, 0, :], xts[:], wct_s[:, 0, :],
                             start=True, stop=True)
            y1sb = til.tile([128, 256], BF16, tag="y1sb")
            nc.vector.tensor_copy(y1sb[:], py1[:, 0, :])
            nc.sync.dma_start(y1d[a * 128 : (a + 1) * 128, :], y1sb[:])

        # ------------------------------------------------------------------
        # Phase A, software-pipelined
        # ------------------------------------------------------------------
        def chunk_front(c):
            st = dict(c=c)
            y1g = gat.tile([128, 1, 256], BF16, tag="y1g")
            nc.gpsimd.dma_gather(
                y1g[:], y1d[:], y1i_s[:, c * 8 : (c + 1) * 8],
                TW, r128, DOUT, transpose=False)
            xdT = gat.tile([128, 1, TB], BF16, tag="xdT")
            nc.gpsimd.dma_gather(
                xdT[:], x16[:], e1i_s[:, c * 32 : (c + 1) * 32],
                TB, r512, DIN, transpose=True)
            gs_s = stt.tile([128, TB], BF16, tag="gs_s")
            nc.sync.dma_start(gs_s[:], gsd[c, :, :])
            madd_s = stt.tile([128, 4, TW], F32, tag="madd")
            nc.sync.dma_start(madd_s[:], madd[c, :, :, :])
            ind_s = stt.tile([128, 4, TW], BF16, tag="ind")
            nc.sync.dma_start(ind_s[:], indd[c, :, :, :])
            indt_s = stt.tile([128, 4, TW], BF16, tag="indt")
            nc.sync.dma_start(indt_s[:], indt[c, :, :, :])
            st["madd"], st["ind"], st["indT"] = madd_s, ind_s, indt_s

            pex = [None, None]
            for f in range(2):
                p = ps_big.tile([128, TB], F32, tag=f"big{f}")
                nc.tensor.matmul(
                    p[:], y1g[:, 0, f * 128 : (f + 1) * 128], gs_s[:],
                    start=True, stop=False)
                nc.tensor.matmul(
                    p[:], wct_s[:, 1, f * 128 : (f + 1) * 128],
                    xdT[:, 0, :], start=False, stop=True)
                pex[f] = p
            st["pex"] = pex
            return st

        def chunk_mid(st):
            c = st["c"]
            exT = st["exT"]
            qT = [None, None]
            for f in range(2):
                pq = ps_big.tile([128, TB], F32, tag=f"big{f}")
                for i in range(2):
                    nc.tensor.matmul(
                        pq[:], wqt_s[:, i, f * 128 : (f + 1) * 128],
                        exT[i][:], start=(i == 0), stop=(i == 1))
                q = act.tile([128, TB], BF16, tag=f"q{f}")
                nc.vector.tensor_scalar_add(q[:], pq[:], bq_s[:, f : f + 1])
                qT[f] = q
            st["qT"] = qT
            khz = khz_bufs[c % 2]
            for f in range(2):
                pk = ps_big.tile([128, TB], F32, tag=f"big{f}")
                for i in range(2):
                    nc.tensor.matmul(
                        pk[:], wkt_s[:, i, f * 128 : (f + 1) * 128],
                        exT[i][:], start=(i == 0), stop=(i == 1))
                for hh in range(2):
                    lo = 64 * hh
                    nc.vector.tensor_scalar_add(
                        khz[f][lo : lo + 64, hh, :], pk[lo : lo + 64, :],
                        bk_s[lo : lo + 64, f : f + 1])
            st["khz"] = khz
            vsb = []
            for tp in range(2):
                pv_ = ps_big.tile([128, 2, 256], F32, tag=f"big{tp}")
                for tt in range(2):
                    t = 2 * tp + tt
                    tsl = slice(t * 128, (t + 1) * 128)
                    for i in range(2):
                        nc.tensor.matmul(
                            pv_[:, tt, :], exT[i][:, tsl], wvt_s[:, i, :],
                            start=(i == 0), stop=(i == 1))
                vp = act.tile([128, 2, 256], BF16, tag=f"vsb{tp}",
                              name=f"vsb{tp}")
                nc.vector.tensor_copy(vp[:], pv_[:])
                vsb.extend([vp[:, 0, :], vp[:, 1, :]])
            st["vsb"] = vsb
            # S + additive mask -> one merged SBUF f32 tile (frees PSUM
            # before the single batched exp); 2 tiles share one score psum
            sm = act.tile([128, 4, H, 128], F32, tag="sm")
            for t in range(4):
                tsl = slice(t * 128, (t + 1) * 128)
                psc = ps_s.tile([128, 4, 128], F32, tag="s")
                for hp in range(2):
                    nc.tensor.matmul(
                        psc[:, 2 * hp : 2 * hp + 2, :],
                        qT[hp][:, tsl], khz[hp][:, :, tsl],
                        start=True, stop=True)
                nc.vector.tensor_tensor(
                    sm[:, t, :, :], psc[:],
                    st["madd"][:, t : t + 1, :].to_broadcast([128, 4, 128]),
                    OP.add)
            st["sm"] = sm

        def chunk_attn(st):
            vsb = st["vsb"]
            pexp = st["pexp"]          # [128, 4, H, 128] bf16
            rsum = til.tile([128, 4 * H], F32, tag="rsum")
            nc.vector.tensor_reduce(
                rsum[:], pexp[:].rearrange("p t h k -> p (t h) k"),
                mybir.AxisListType.X, OP.add)
            rrec = til.tile([128, 4 * H], F32, tag="rrec")
            nc.vector.reciprocal(rrec[:], rsum[:])
            rrec16 = til.tile([128, 4 * H], BF16, tag="rrec16")
            nc.vector.tensor_copy(rrec16[:], rrec[:])
            nc.vector.tensor_tensor(
                pexp[:].rearrange("p t h k -> p (t h) k"),
                pexp[:].rearrange("p t h k -> p (t h) k"),
                rrec16[:].rearrange("p (q o) -> p q o", o=1
                                    ).to_broadcast([128, 4 * H, 128]),
                OP.mult)
            posb = stt.tile([128, 4, 256], F32, tag="posb")
            po = None
            for t in range(4):
                if t % 2 == 0:
                    po = ps_o.tile([128, 2, 256], F32, tag="po")
                pot = po[:, t % 2, :]
                ppt = ps_t.tile([128, H, 128], BF16, tag="ppt")
                pvo = ps_pv.tile([64, H, 128], F32, tag="pvo")
                for h in range(H):
                    nc.tensor.transpose(ppt[:, h, :], pexp[:, t, h, :],
                                        ident[:])
                    pts = til.tile([128, 128], BF16, tag=f"pts{h % 2}",
                                   name=f"pts{h % 2}")
                    nc.scalar.activation(pts[:], ppt[:, h, :], AF.Identity)
                    nc.tensor.matmul(
                        pvo[:, h, :], vsb[t][:, 64 * h : 64 * h + 64], pts[:],
                        start=True, stop=True)
                ao = til.tile([64, H, 128], BF16, tag="aoall", name="aoall")
                nc.vector.tensor_copy(ao[:], pvo[:])
                for h in range(H):
                    nc.tensor.matmul(pot, ao[:, h, :], wot_s[:, h, :],
                                     start=(h == 0), stop=False)
                nc.tensor.matmul(pot, ones1[:, :128], bo_s[:],
                                 start=False, stop=True)
                if t % 2 == 1:
                    nc.vector.tensor_copy(
                        posb[:, t - 1 : t + 1, :], po[:])
            st["po_sb"] = posb
            st["hsb"] = stt.tile([128, 4, 256], F32, tag="hsb", name="hsb")

        def chunk_gates(st):
            hview = st["hsb"][:].rearrange(
                "p t (h c j) -> p t h c j", h=H, c=2)
            lg = til.tile([128, 4, H], F32, tag="lg")
            nc.vector.tensor_reduce(
                lg[:], hview[:, :, :, 1, :], mybir.AxisListType.X, OP.add)
            st["lg"] = lg

        def chunk_tail(st):
            c = st["c"]
            ind_s, indt_s = st["ind"], st["indT"]
            ew = st["ew"]              # [128, 4, H] f32
            ew16 = til.tile([128, 4, H], BF16, tag="ew16")
            nc.vector.tensor_copy(ew16[:], ew[:])
            smt = ps_o.tile([128, 2, 256], F32, tag="po", name="smt")
            smtv = smt[:].rearrange("p a (t h) -> p a t h", t=64)
            for t in range(4):
                nc.tensor.matmul(smtv[:, 0, t, :], ind_s[:, t, :],
                                 ew16[:, t, :], start=True, stop=True)
            dne = til.tile([128, 4, H], F32, tag="dne")
            nc.vector.tensor_scalar_add(dne[:], smtv[:, 0, :4, :], 1e-20)
            dnr = til.tile([128, 4, H], F32, tag="dnr")
            nc.vector.reciprocal(dnr[:], dne[:])
            dnr16 = til.tile([128, 4, H], BF16, tag="dnr16")
            nc.vector.tensor_copy(dnr16[:], dnr[:])
            for t in range(4):
                nc.tensor.matmul(smtv[:, 1, t, :], indt_s[:, t, :],
                                 dnr16[:, t, :], start=True, stop=True)
            al = til.tile([128, 4, H], F32, tag="al")
            nc.vector.tensor_tensor(al[:], ew[:], smtv[:, 1, :4, :], OP.mult)
            hview = st["hsb"][:].rearrange(
                "p t (h c j) -> p t h c j", h=H, c=2)
            ct_chunk = stt.tile([128, 4, 128], BF16, tag="ct")
            nc.vector.tensor_tensor(
                ct_chunk[:].rearrange("p t (h j) -> p t h j", h=H),
                hview[:, :, :, 0, :],
                al[:].rearrange("p t (h o) -> p t h o", o=1
                                ).to_broadcast([128, 4, H, 32]),
                OP.mult)
            nc.sync.dma_start(
                scratch[c * TB : (c + 1) * TB, :].rearrange(
                    "(b p) d -> p b d", p=128),
                ct_chunk[:])

        def emit_gelu_block(prev, st):
            if prev is not None:
                nc.scalar.activation(
                    prev["hsb"][:], prev["po_sb"][:], AF.Gelu)
                chunk_gates(prev)
            exT = [None, None]
            for f in range(2):
                ex = act.tile([128, TB], BF16, tag=f"ex{f}")
                nc.scalar.activation(ex[:], st["pex"][f][:], AF.Gelu,
                                     bias=bc_s[:, f : f + 1], scale=1.0)
                exT[f] = ex
            st["exT"] = exT

        def emit_exp_block(prev, st):
            if prev is not None:
                ew = til.tile([128, 4, H], F32, tag="ew")
                nc.scalar.activation(ew[:], prev["lg"][:], AF.Exp,
                                     scale=1.0 / 32.0)
                prev["ew"] = ew
            pexp = act.tile([128, 4, H, 128], BF16, tag="pexp")
            nc.scalar.activation(pexp[:], st["sm"][:], AF.Exp)
            st["pexp"] = pexp

        dbg_name = DEBUG[0]
        dbgf, dbgh = T.get("dbgf"), T.get("dbgh")

        def emit_dbg(st):
            c = st["c"]
            if dbg_name == "sm":
                nc.sync.dma_start(
                    dbgf[c, :, :], st["sm"][:].rearrange("p a h k -> p (a h k)"))
            elif dbg_name == "posb":
                nc.sync.dma_start(
                    dbgf[c, :, :1024],
                    st["po_sb"][:].rearrange("p a k -> p (a k)"))
            elif dbg_name == "hsb":
                nc.sync.dma_start(
                    dbgf[c, :, :1024],
                    st["hsb"][:].rearrange("p a k -> p (a k)"))
            elif dbg_name == "pexp":
                nc.sync.dma_start(
                    dbgh[c, :, :], st["pexp"][:].rearrange("p a h k -> p (a h k)"))
            elif dbg_name == "ex":
                nc.sync.dma_start(dbgh[c, :, :512], st["exT"][0][:])
                nc.sync.dma_start(dbgh[c, :, 512:1024], st["exT"][1][:])
            elif dbg_name == "q":
                nc.sync.dma_start(dbgh[c, :, :512], st["qT"][0][:])
                nc.sync.dma_start(dbgh[c, :, 512:1024], st["qT"][1][:])
            elif dbg_name == "k":
                khz = st["khz"]
                nc.sync.dma_start(
                    dbgh[c, :, :1024], khz[0][:].rearrange("p a k -> p (a k)"))
                nc.sync.dma_start(
                    dbgh[c, :, 1024:], khz[1][:].rearrange("p a k -> p (a k)"))

        prev = None
        for c in range(NCH):
            st = chunk_front(c)
            emit_gelu_block(prev, st)
            chunk_mid(st)
            emit_exp_block(prev, st)
            if prev is not None:
                chunk_tail(prev)
            chunk_attn(st)
            if dbg_name in ("sm", "pexp", "ex", "q", "k", "posb"):
                emit_dbg(st)
            elif dbg_name == "hsb" and prev is not None:
                emit_dbg(prev)
            prev = st

        # epilogue for last chunk
        nc.scalar.activation(prev["hsb"][:], prev["po_sb"][:], AF.Gelu)
        chunk_gates(prev)
        ew = til.tile([128, 4, H], F32, tag="ew")
        nc.scalar.activation(ew[:], prev["lg"][:], AF.Exp, scale=1.0 / 32.0)
        prev["ew"] = ew
        chunk_tail(prev)

        # ------------------------------------------------------------------
        # Phase S: transposed gather of e1-sorted contribs -> window seg-sum
        # ------------------------------------------------------------------
        psA.close()   # free phase-A PSUM banks
        spool = ctx.enter_context(tc.tile_pool(name="sp", bufs=2))
        ps_w = ctx.enter_context(tc.tile_pool(name="psw", bufs=2, space="PSUM"))
        ps_t2 = ctx.enter_context(tc.tile_pool(name="pst2", bufs=2, space="PSUM"))

        def emit_gather(ci):
            n = TB if ci < NSCH - 1 else last_n
            reg = r512 if ci < NSCH - 1 else rlast
            cg = spool.tile([128, 1, TB], BF16, tag="cg")
            nc.gpsimd.dma_gather(
                cg[:, :, :n], scratch[:],
                sgi_s[:, ci * 32 : ci * 32 + n // 16],
                n, reg, 128, transpose=True)
            iw = spool.tile([128, 4, TW], BF16, tag="iw")
            nt = min(4, NST - ci * 4)
            nc.sync.dma_start(
                iw[:, :nt, :],
                sindw[ci * 4 : ci * 4 + nt, :, :].rearrange("t s d -> s t d"))
            return cg, iw

        tix = 0
        cur = None
        for w in range(NW):
            rows = min(WIN, N - w * WIN)
            pw = ps_w.tile([128, 128], F32, tag=f"w{w % 2}")
            for j in range(Tws[w]):
                ci, j2 = tix // 4, tix % 4
                if j2 == 0 or cur is None:
                    cur = emit_gather(ci)
                cg, iw = cur
                ppt = ps_t2.tile([128, 128], BF16, tag=f"ppt{tix % 2}")
                nc.tensor.transpose(
                    ppt[:], cg[:, 0, j2 * 128 : (j2 + 1) * 128], ident[:])
                cem = spool.tile([128, 128], BF16, tag=f"cem{tix % 2}")
                nc.scalar.activation(cem[:], ppt[:], AF.Identity)
                nc.tensor.matmul(pw[:], iw[:, j2, :], cem[:],
                                 start=(j == 0), stop=(j == Tws[w] - 1))
                tix += 1
            wout = spool.tile([128, 128], BF16, tag=f"wout{w % 2}")
            nc.vector.tensor_copy(wout[:], pw[:])
            nc.sync.dma_start(partial[w * WIN : w * WIN + rows, :],
                              wout[:rows, :])
        assert tix == NST

        # ------------------------------------------------------------------
        # Phase R: ReduceScatter over 8 cores, write this core's shard
        # ------------------------------------------------------------------
        nc.gpsimd.collective_compute(
            "ReduceScatter", mybir.AluOpType.add,
            replica_groups=[list(range(NCORES))],
            ins=[partial[:]], outs=[rsout[:]])
        ob = spool.tile([125, 8, 128], BF16, tag="ob")
        nc.sync.dma_start(ob[:], rsout[:].rearrange("(a p) d -> p a d", p=125))
        obf = spool.tile([125, 8, 128], F32, tag="obf")
        nc.vector.tensor_copy(obf[:], ob[:])
        nc.sync.dma_start(
            outp[:NPC, :].rearrange("(a p) d -> p a d", p=125), obf[:])


# ----------------------------------------------------------------------------
# entry point
# ----------------------------------------------------------------------------

def kernel(x, Wc, bc, Wq, bq, Wk, bk, Wv, bv, Wo, bo, e0, e1, pos, max_deg):
    assert int(max_deg) == MAXD and x.shape == (N, DIN)
    in_maps, dims = _prepare(
        np.asarray(x, np.float32), np.asarray(Wc), np.asarray(bc),
        np.asarray(Wq), np.asarray(bq), np.asarray(Wk), np.asarray(bk),
        np.asarray(Wv), np.asarray(bv), np.asarray(Wo), np.asarray(bo),
        e0, e1)
    nc = _build(dims)
    res = run_bass_kernel_spmd(
        nc, in_maps, core_ids=list(range(NCORES)), trace=TRACE[0])
    LAST_PROFILE.clear()
    LAST_PROFILE.update(dict(
        exec_time_ns=res.exec_time_ns,
        trace=res.instructions_and_trace,
        profile_json=res.profile_json,
    ))
    out = np.concatenate(
        [res.results[k]["out"][:NPC] for k in range(NCORES)], axis=0)
    return out.astype(np.float32)
